# revision 17
# baseline (speedup 1.0000x reference)
"""Tensor-parallel causal attention kernel for 8 trn2 NeuronCores.

Problem: B=2, S=2048, H=2048, 16 heads, head_dim=128 fp32.
  qkv = hidden @ w_qkv.T ; causal attention ; out = attn @ w_o.T

Sharding (hardcoded): core c in 0..7 handles batch b=c//4 and heads
hs = [4*(c%4) .. 4*(c%4)+3].  Each core computes a partial o_proj
output (contraction over its 512 hidden dims); the host sums the 4
partials per batch and transposes.  No device collectives.

Device-side layout (all host-pretiled to partition-major [128, ...]):
  xt  [128,16,2048] f32 : xt[p,ko,s]  = hidden[b, s, ko*128+p]
  wq  [128,16, 512] f32 : wq[p,ko,o]  = w_qkv[q_rows[o],  ko*128+p]
  wk  [128,16, 512] f32 : wk[p,ko,o]  = w_qkv[k_rows[o],  ko*128+p]
  wv  [128,16, 512] f32 : wv[p,ko,d]  = w_qkv[v_rows[d],  ko*128+p]
  wo  [128, 4,2048] f32 : wo[p,kb,o]  = w_o[o, cols[kb*128+p]]
  outt[128,16,2048] f32 : outt[p,ot,s] = outT_partial[ot*128+p, s]

Toolchain quirks this code works around (walrus 1-sync-wait slots):
  - chunked tail drain monkeypatch
  - fp32r operands produced by compute ops (ACT/DVE), same-engine pairing
  - tiny "observer"/"dummy-matmul" ops so any instruction needs at most
    one fresh semaphore
"""
import numpy as np

import concourse.bass as bass
import concourse.mybir as mybir
import concourse.tile as tile
from concourse.bass_utils import run_bass_kernel_spmd
from concourse.vector_clock import ScopedClock, VectorClock

P = 128
S = 2048
H = 2048
NH_LOCAL = 4          # heads per core
KO = H // P           # 16 contraction chunks for the projections
SQ = 512              # q chunk width
NQC = S // SQ         # 4 q chunks
NKB = S // P          # 16 key blocks
F32 = mybir.dt.float32
F32R = mybir.dt.float32r
AF = mybir.ActivationFunctionType
SCALE = 1.0 / float(np.sqrt(128.0))

XCH = 256             # x chunk width in phase 1
NXCH = S // XCH       # 8 chunks


def _drain_and_barrier_chunked(self, tick_clock, wait_clock, _MAX=1):
    """Split the kernel-tail drain's waits: walrus allows only one sync
    wait per instruction in this toolchain."""
    g = tick_clock.global_clock
    n = len(g)
    vals = [g[i] for i in range(n)]
    nz = [i for i, v in enumerate(vals) if v > 0]
    chunks = [nz[i:i + _MAX] for i in range(0, len(nz), _MAX)] or [[]]
    for chunk in chunks:
        vec = [vals[i] if i in chunk else 0 for i in range(n)]
        d = self.nc.sync.drain()
        wait_clock.add_sem_waits(d.ins, ScopedClock({None: VectorClock(vec)}))
    self.nc.all_engine_barrier()
    assert self.sems is not None
    popped = self.nc._tile_sem_poison_stack.pop()
    assert popped is self._sem_poison
    self.nc.clear_and_free_semaphores(list(self.sems.allocated().values()))
    self.nc.all_engine_barrier()


tile.TileContext._drain_and_barrier = _drain_and_barrier_chunked


def _split_multi_waits(nc):
    """walrus allows ONE sync wait per instruction: hoist extra waits onto
    same-engine NoOps inserted directly before the offending instruction
    (identical semantics — the engine queue blocks on each in turn)."""
    ctr = 0
    for f in nc.m.functions:
        for blk in f.blocks:
            new = []
            changed = False
            for inst in blk.instructions:
                si = inst.sync_info
                waits = list(si.on_wait) if si and si.on_wait else []
                if len(waits) > 1:
                    changed = True
                    for w in waits[:-1]:
                        ctr += 1
                        nop = mybir.InstNoOp(name=f"I-wsplit-{ctr}",
                                             engine=inst.engine,
                                             ins=[], outs=[])
                        nop.sync_info = mybir.SyncInfo(on_wait=[w],
                                                       on_update=[])
                        new.append(nop)
                    ups = list(si.on_update) if si.on_update else []
                    inst.sync_info = mybir.SyncInfo(on_wait=[waits[-1]],
                                                    on_update=ups)
                new.append(inst)
            if changed:
                blk.instructions = new
    return ctr


def build():
    nc = bass.Bass()
    xt = nc.dram_tensor("xt", [P, KO, S], F32, kind="ExternalInput")
    wq = nc.dram_tensor("wq", [P, KO, NH_LOCAL * P], F32, kind="ExternalInput")
    wk = nc.dram_tensor("wk", [P, KO, NH_LOCAL * P], F32, kind="ExternalInput")
    wv = nc.dram_tensor("wv", [P, KO, NH_LOCAL * P], F32, kind="ExternalInput")
    wo = nc.dram_tensor("wo", [P, NH_LOCAL, S], F32, kind="ExternalInput")
    outt = nc.dram_tensor("outt", [P, KO, S], F32, kind="ExternalOutput")

    with tile.TileContext(nc) as tc:
        from contextlib import ExitStack
        with ExitStack() as ctx:
            const = ctx.enter_context(tc.tile_pool(name="const", bufs=1))

            # ---- constants -------------------------------------------------
            ones_f = const.tile([P, 1], F32)
            nc.vector.memset(ones_f[:], 1.0)
            ones_r = const.tile([P, 1], F32R)
            nc.scalar.copy(ones_r[:], ones_f[:])
            onesrow_f = const.tile([1, P], F32)
            nc.vector.memset(onesrow_f[:], 1.0)
            onesrow_r = const.tile([1, P], F32R)
            nc.scalar.copy(onesrow_r[:], onesrow_f[:])
            ones16_f = const.tile([P, 16], F32)
            nc.vector.memset(ones16_f[:], 1.0)
            ones16_r = const.tile([P, 16], F32R)
            nc.scalar.copy(ones16_r[:], ones16_f[:])
            # observer scratch tiles (one per engine that needs them)
            obs_act = const.tile([1, 1], F32)
            nc.vector.memset(obs_act[:], 0.0)
            obs_dve = const.tile([1, 1], F32)
            nc.vector.memset(obs_dve[:], 0.0)

            def _one(ap):
                return ap[tuple(slice(0, 1) for _ in ap.shape)]

            # WAR-absorbers: a tiny write into a to-be-reused slot makes the
            # writing engine observe the slot's previous readers' semaphore,
            # so the real (full) write right after needs only its RAW sem.
            def act_war_touch(ap):
                nc.scalar.copy(_one(ap[:]), obs_act[:])

            def dve_war_touch(ap):
                nc.vector.tensor_copy(_one(ap[:]), obs_dve[:])

            def dummy_mm(ps_tile):
                # tiny PE op to absorb this psum slot's release semaphore
                nc.tensor.matmul(ps_tile[0:1, 0:16], ones_r[:], ones16_r[:],
                                 start=True, stop=True, skip_group_check=True)

            # ---- residents (phase 1+2, freed before phase 3) --------------
            qkv_pool = ctx.enter_context(tc.tile_pool(name="qkvp", bufs=1))
            # Q,K as qkvT: [d_in, o_tile(0-3 Q heads, 4-7 K heads), s]
            qk_sb = qkv_pool.tile([P, 2 * NH_LOCAL, S], F32R)
            # V as [s_in, s_tile, d_local]
            v_sb = qkv_pool.tile([P, NKB, NH_LOCAL * P], F32R)

            # ================= phase 1: QKV projection =====================
            with tc.tile_pool(name="p1wl", bufs=1) as p1wl, \
                 tc.tile_pool(name="p1w", bufs=1) as p1w, \
                 tc.tile_pool(name="p1xl", bufs=1) as p1xl, \
                 tc.tile_pool(name="p1x", bufs=2) as p1x, \
                 tc.tile_pool(name="p1ps", bufs=4, space="PSUM") as p1ps:

                for pss, wdram in (("q", wq), ("k", wk), ("v", wv)):
                    w_r = p1w.tile([P, KO, NH_LOCAL * P], F32R, tag="wr")
                    act_war_touch(w_r)  # absorb PE readers of previous pass
                    for wc in range(4):
                        w_land = p1wl.tile([P, KO, P], F32, tag="wland")
                        nc.sync.dma_start(
                            w_land[:], wdram.ap()[:, :, wc * P:(wc + 1) * P])
                        nc.scalar.copy(w_r[:, :, wc * P:(wc + 1) * P],
                                       w_land[:])

                    for xc in range(NXCH):
                        x_land = p1xl.tile([P, KO, XCH], F32, tag="xland")
                        nc.sync.dma_start(
                            x_land[:], xt.ap()[:, :, xc * XCH:(xc + 1) * XCH])
                        x_r = p1x.tile([P, KO, XCH], F32R, tag="xr")
                        dve_war_touch(x_r)
                        nc.vector.tensor_copy(x_r[:], x_land[:])

                        if pss in ("q", "k"):
                            ot_base = 0 if pss == "q" else NH_LOCAL
                            for ot in range(NH_LOCAL):
                                ps = p1ps.tile([P, XCH], F32, tag="p1qk")
                                dummy_mm(ps)
                                for k in range(KO):
                                    nc.tensor.matmul(
                                        ps[:], w_r[:, k, ot * P:(ot + 1) * P],
                                        x_r[:, k], start=(k == 0),
                                        stop=(k == KO - 1))
                                nc.scalar.copy(
                                    qk_sb[:, ot_base + ot,
                                          xc * XCH:(xc + 1) * XCH], ps[:])
                        else:
                            # V pass: out [s_tile(128), d(512)]
                            for st in range(XCH // P):
                                stg = xc * (XCH // P) + st
                                ps = p1ps.tile([P, NH_LOCAL * P], F32,
                                               tag="p1v")
                                dummy_mm(ps)
                                for k in range(KO):
                                    nc.tensor.matmul(
                                        ps[:],
                                        x_r[:, k, st * P:(st + 1) * P],
                                        w_r[:, k], start=(k == 0),
                                        stop=(k == KO - 1))
                                nc.scalar.copy(v_sb[:, stg, :], ps[:])

            # ================= phase 2: attention ==========================
            # per (head, q_chunk): ST = K_blk @ Q^T; est = exp(ST*scale);
            # causal mask on diagonal blocks; sums += ones@est;
            # attn += V_blk^T... (lhsT=V_blk) ; normalize via rank-1 recip.
            attn_pool = ctx.enter_context(tc.tile_pool(name="attnp", bufs=1))
            attnT = attn_pool.tile([P, NH_LOCAL, S], F32R)

            with tc.tile_pool(name="p2sb", bufs=1) as p2sb, \
                 tc.tile_pool(name="p2est", bufs=4) as p2est, \
                 tc.tile_pool(name="p2st", bufs=2, space="PSUM") as p2st, \
                 tc.tile_pool(name="p2at", bufs=2, space="PSUM") as p2at, \
                 tc.tile_pool(name="p2sm", bufs=2, space="PSUM") as p2sm, \
                 tc.tile_pool(name="p2rep", bufs=2, space="PSUM") as p2rep:

                for h in range(NH_LOCAL):
                    for qc in range(NQC):
                        nkb = 4 * (qc + 1)
                        qs = qc * SQ

                        at_ps = p2at.tile([P, SQ], F32, tag="atps")
                        dummy_mm(at_ps)
                        sm_ps = p2sm.tile([1, SQ], F32, tag="smps")
                        dummy_mm(sm_ps)

                        for kb in range(nkb):
                            st_ps = p2st.tile([P, SQ], F32, tag="stps")
                            nc.tensor.matmul(
                                st_ps[:],
                                qk_sb[:, NH_LOCAL + h, kb * P:(kb + 1) * P],
                                qk_sb[:, h, qs:qs + SQ],
                                start=True, stop=True)
                            est = p2est.tile([P, SQ], F32R, tag="est")
                            nc.scalar.activation(est[:], st_ps[:], AF.Exp,
                                                 scale=SCALE)
                            if kb * P + P - 1 > qs:  # crosses the diagonal
                                nc.gpsimd.affine_select(
                                    est[:], est[:], [[1, SQ]],
                                    mybir.AluOpType.is_ge, 0.0,
                                    base=qs - kb * P, channel_multiplier=-1)
                            nc.tensor.matmul(sm_ps[:], ones_r[:], est[:],
                                             start=(kb == 0),
                                             stop=(kb == nkb - 1))
                            nc.tensor.matmul(
                                at_ps[:],
                                v_sb[:, kb, h * P:(h + 1) * P], est[:],
                                start=(kb == 0), stop=(kb == nkb - 1))

                        logs = p2sb.tile([1, SQ], F32, tag="logs")
                        nc.scalar.activation(logs[:], sm_ps[:], AF.Ln)
                        recip = p2sb.tile([1, SQ], F32R, tag="recip")
                        nc.scalar.activation(recip[:], logs[:], AF.Exp,
                                             scale=-1.0)
                        rep_ps = p2rep.tile([P, SQ], F32, tag="repps")
                        dummy_mm(rep_ps)
                        nc.tensor.matmul(rep_ps[:], onesrow_r[:], recip[:],
                                         start=True, stop=True)
                        rep_sb = p2sb.tile([P, SQ], F32, tag="repsb")
                        nc.vector.tensor_copy(rep_sb[:], rep_ps[:])
                        nc.vector.tensor_mul(attnT[:, h, qs:qs + SQ],
                                             at_ps[:], rep_sb[:])

            # ================= phase 3: o_proj partial =====================
            with tc.tile_pool(name="p3wl", bufs=1) as p3wl, \
                 tc.tile_pool(name="p3w", bufs=1) as p3w, \
                 tc.tile_pool(name="p3sb", bufs=2) as p3sb, \
                 tc.tile_pool(name="p3ps", bufs=4, space="PSUM") as p3ps:
                wo_r = p3w.tile([P, NH_LOCAL, S], F32R)
                for wc in range(4):
                    wo_land = p3wl.tile([P, NH_LOCAL, SQ], F32, tag="woland")
                    nc.sync.dma_start(
                        wo_land[:], wo.ap()[:, :, wc * SQ:(wc + 1) * SQ])
                    nc.scalar.copy(wo_r[:, :, wc * SQ:(wc + 1) * SQ],
                                   wo_land[:])
                # absorb {ACT wo_r} once so first real MM needs only attnT
                p3dummy = p3ps.tile([1, 16], F32, tag="p3dummy",
                                    name="p3dummy")
                nc.tensor.matmul(p3dummy[:], wo_r[:, 0, 0:1], ones16_r[:],
                                 start=True, stop=True, skip_group_check=True)
                for ot in range(KO):
                    stage = p3sb.tile([P, S], F32, tag="p3stage")
                    act_war_touch(stage)
                    for sc in range(NQC):
                        ps = p3ps.tile([P, SQ], F32, tag="p3ps")
                        dummy_mm(ps)
                        for kb in range(NH_LOCAL):
                            nc.tensor.matmul(
                                ps[:], wo_r[:, kb, ot * P:(ot + 1) * P],
                                attnT[:, kb, sc * SQ:(sc + 1) * SQ],
                                start=(kb == 0), stop=(kb == NH_LOCAL - 1))
                        nc.scalar.copy(stage[:, sc * SQ:(sc + 1) * SQ], ps[:])
                    nc.sync.dma_start(outt.ap()[:, ot, :], stage[:])
    _split_multi_waits(nc)
    return nc


_NC_CACHE = None


def _get_nc():
    global _NC_CACHE
    if _NC_CACHE is None:
        _NC_CACHE = build()
    return _NC_CACHE


def _prep_inputs(hidden_states, w_qkv, w_o):
    """Host-side shard + pre-tile for the 8 cores."""
    hidden_states = np.asarray(hidden_states, dtype=np.float32)
    w_qkv = np.asarray(w_qkv, dtype=np.float32)
    w_o = np.asarray(w_o, dtype=np.float32)
    B = hidden_states.shape[0]

    in_maps = []
    xt_by_b = {}
    for b in range(B):
        # xt[p, ko, s] = hidden[b, s, ko*128+p]
        xt = np.ascontiguousarray(
            hidden_states[b].T.reshape(KO, P, S).transpose(1, 0, 2))
        xt_by_b[b] = xt
    for c in range(8):
        b = c // 4
        hs = [4 * (c % 4) + j for j in range(NH_LOCAL)]
        q_rows = np.concatenate([np.arange(h * P, (h + 1) * P) for h in hs])
        k_rows = q_rows + H
        v_rows = q_rows + 2 * H

        def wtile(rows):
            # [p, ko, o] = w_qkv[rows[o], ko*128+p]
            w = w_qkv[rows, :]                      # [512, 2048]
            return np.ascontiguousarray(
                w.T.reshape(KO, P, len(rows)).transpose(1, 0, 2))

        # wo[p, kb, o] = w_o[o, cols[kb*128+p]]
        wo_c = np.ascontiguousarray(
            w_o[:, q_rows].T.reshape(NH_LOCAL, P, S).transpose(1, 0, 2))
        in_maps.append({
            "xt": xt_by_b[b],
            "wq": wtile(q_rows),
            "wk": wtile(k_rows),
            "wv": wtile(v_rows),
            "wo": wo_c,
        })
    return in_maps


def run(hidden_states, w_qkv, w_o, trace=False, trace_cores=None):
    in_maps = _prep_inputs(hidden_states, w_qkv, w_o)
    nc = _get_nc()
    kwargs = {}
    if trace:
        kwargs["trace_cores"] = (trace_cores if trace_cores is not None
                                 else list(range(8)))
    res = run_bass_kernel_spmd(nc, in_maps, core_ids=list(range(8)),
                               trace=trace, **kwargs)
    B, S_, H_ = np.asarray(hidden_states).shape
    out = np.zeros((B, S_, H_), dtype=np.float32)
    for c in range(8):
        b = c // 4
        outt = res.results[c]["outt"]               # [128, 16, 2048]
        outT = outt.transpose(1, 0, 2).reshape(H_, S_)
        out[b] += outT.T
    return out, res


def kernel(hidden_states, w_qkv, w_o):
    out, _ = run(hidden_states, w_qkv, w_o, trace=False)
    return out


# revision 26
# speedup vs baseline: 1.3085x; 1.3085x over previous
"""Tensor-parallel causal attention kernel for 8 trn2 NeuronCores.

Problem: B=2, S=2048, H=2048, 16 heads, head_dim=128 fp32.
  qkv = hidden @ w_qkv.T ; causal attention ; out = attn @ w_o.T

Sharding (hardcoded): core c in 0..7 handles batch b=c//4 and heads
hs = [4*(c%4) .. 4*(c%4)+3].  Each core computes a partial o_proj
output (contraction over its 512 hidden dims); the host sums the 4
partials per batch and transposes.  No device collectives.

Device-side layout (all host-pretiled to partition-major [128, ...]):
  xt  [128,16,2048] f32 : xt[p,ko,s]  = hidden[b, s, ko*128+p]
  wq  [128,16, 512] f32 : wq[p,ko,o]  = w_qkv[q_rows[o],  ko*128+p]
  wk  [128,16, 512] f32 : wk[p,ko,o]  = w_qkv[k_rows[o],  ko*128+p]
  wv  [128,16, 512] f32 : wv[p,ko,d]  = w_qkv[v_rows[d],  ko*128+p]
  wo  [128, 4,2048] f32 : wo[p,kb,o]  = w_o[o, cols[kb*128+p]]
  outt[128,16,2048] f32 : outt[p,ot,s] = outT_partial[ot*128+p, s]

Toolchain quirks this code works around (walrus 1-sync-wait slots):
  - chunked tail drain monkeypatch
  - fp32r operands produced by compute ops (ACT/DVE), same-engine pairing
  - tiny "observer"/"dummy-matmul" ops so any instruction needs at most
    one fresh semaphore
"""
import numpy as np

import concourse.bass as bass
import concourse.mybir as mybir
import concourse.tile as tile
from concourse.bass_utils import run_bass_kernel_spmd
from concourse.vector_clock import ScopedClock, VectorClock

P = 128
S = 2048
H = 2048
NH_LOCAL = 4          # heads per core
KO = H // P           # 16 contraction chunks for the projections
SQ = 512              # q chunk width
NQC = S // SQ         # 4 q chunks
NKB = S // P          # 16 key blocks
F32 = mybir.dt.float32
F32R = mybir.dt.float32r
BF = mybir.dt.bfloat16
AF = mybir.ActivationFunctionType
SCALE = 1.0 / float(np.sqrt(128.0))

XCH = 512             # x chunk width in phase 1
NXCH = S // XCH       # 4 chunks


def _drain_and_barrier_chunked(self, tick_clock, wait_clock, _MAX=1):
    """Split the kernel-tail drain's waits: walrus allows only one sync
    wait per instruction in this toolchain."""
    g = tick_clock.global_clock
    n = len(g)
    vals = [g[i] for i in range(n)]
    nz = [i for i, v in enumerate(vals) if v > 0]
    chunks = [nz[i:i + _MAX] for i in range(0, len(nz), _MAX)] or [[]]
    for chunk in chunks:
        vec = [vals[i] if i in chunk else 0 for i in range(n)]
        d = self.nc.sync.drain()
        wait_clock.add_sem_waits(d.ins, ScopedClock({None: VectorClock(vec)}))
    self.nc.all_engine_barrier()
    assert self.sems is not None
    popped = self.nc._tile_sem_poison_stack.pop()
    assert popped is self._sem_poison
    self.nc.clear_and_free_semaphores(list(self.sems.allocated().values()))
    self.nc.all_engine_barrier()


tile.TileContext._drain_and_barrier = _drain_and_barrier_chunked


def _split_multi_waits(nc):
    """walrus allows ONE sync wait per instruction: hoist extra waits onto
    same-engine NoOps inserted directly before the offending instruction
    (identical semantics — the engine queue blocks on each in turn)."""
    ctr = 0
    for f in nc.m.functions:
        for blk in f.blocks:
            new = []
            changed = False
            for inst in blk.instructions:
                si = inst.sync_info
                waits = list(si.on_wait) if si and si.on_wait else []
                if len(waits) > 1:
                    changed = True
                    for w in waits[:-1]:
                        ctr += 1
                        nop = mybir.InstNoOp(name=f"I-wsplit-{ctr}",
                                             engine=inst.engine,
                                             ins=[], outs=[])
                        nop.sync_info = mybir.SyncInfo(on_wait=[w],
                                                       on_update=[])
                        new.append(nop)
                    ups = list(si.on_update) if si.on_update else []
                    inst.sync_info = mybir.SyncInfo(on_wait=[waits[-1]],
                                                    on_update=ups)
                new.append(inst)
            if changed:
                blk.instructions = new
    return ctr


def build():
    nc = bass.Bass()
    xt = nc.dram_tensor("xt", [P, KO, S], F32, kind="ExternalInput")
    wq = nc.dram_tensor("wq", [P, KO, NH_LOCAL * P], F32, kind="ExternalInput")
    wk = nc.dram_tensor("wk", [P, KO, NH_LOCAL * P], F32, kind="ExternalInput")
    wv = nc.dram_tensor("wv", [P, KO, NH_LOCAL * P], F32, kind="ExternalInput")
    wo = nc.dram_tensor("wo", [P, NH_LOCAL, S], F32, kind="ExternalInput")
    outt = nc.dram_tensor("outt", [P, KO, S], F32, kind="ExternalOutput")

    with tile.TileContext(nc) as tc:
        from contextlib import ExitStack
        with ExitStack() as ctx:
            const = ctx.enter_context(tc.tile_pool(name="const", bufs=1))

            # ---- constants -------------------------------------------------
            ones_f = const.tile([P, 1], F32)
            nc.vector.memset(ones_f[:], 1.0)
            ones_r = const.tile([P, 1], F32R)
            nc.scalar.copy(ones_r[:], ones_f[:])
            ones_bf = const.tile([P, 1], BF)
            nc.scalar.copy(ones_bf[:], ones_f[:])
            onesrow_f = const.tile([1, P], F32)
            nc.vector.memset(onesrow_f[:], 1.0)
            onesrow_r = const.tile([1, P], F32R)
            nc.scalar.copy(onesrow_r[:], onesrow_f[:])
            ones16_f = const.tile([P, 16], F32)
            nc.vector.memset(ones16_f[:], 1.0)
            ones16_r = const.tile([P, 16], F32R)
            nc.scalar.copy(ones16_r[:], ones16_f[:])
            ones16_bf = const.tile([P, 16], BF)
            nc.scalar.copy(ones16_bf[:], ones16_f[:])
            # observer scratch tiles (one per engine that needs them)
            obs_act = const.tile([1, 1], F32)
            nc.vector.memset(obs_act[:], 0.0)
            obs_dve = const.tile([1, 1], F32)
            nc.vector.memset(obs_dve[:], 0.0)

            def _one(ap):
                return ap[tuple(slice(0, 1) for _ in ap.shape)]

            # WAR-absorbers: a tiny write into a to-be-reused slot makes the
            # writing engine observe the slot's previous readers' semaphore,
            # so the real (full) write right after needs only its RAW sem.
            def act_war_touch(ap):
                nc.scalar.copy(_one(ap[:]), obs_act[:])

            def dve_war_touch(ap):
                nc.vector.tensor_copy(_one(ap[:]), obs_dve[:])

            def dummy_mm(ps_tile):
                # tiny PE op to absorb this psum slot's release semaphore
                nc.tensor.matmul(ps_tile[0:1, 0:16], ones_r[:], ones16_r[:],
                                 start=True, stop=True, skip_group_check=True)

            # ---- residents (phase 1+2, freed before phase 3) --------------
            qkv_pool = ctx.enter_context(tc.tile_pool(name="qkvp", bufs=1))
            # Q,K as qkvT: [d_in, o_tile(0-3 Q heads, 4-7 K heads), s]
            qk_sb = qkv_pool.tile([P, 2 * NH_LOCAL, S], BF)
            # V as [s_in, s_tile, d_local]
            v_sb = qkv_pool.tile([P, NKB, NH_LOCAL * P], BF)

            # ================= phase 1: QKV projection =====================
            # single pass over x with all of w_qkv resident in bf16:
            # w_r dims 0:512 = Q heads, 512:1024 = K heads, 1024:1536 = V
            with tc.tile_pool(name="p1wl", bufs=2) as p1wl, \
                 tc.tile_pool(name="p1w", bufs=1) as p1w, \
                 tc.tile_pool(name="p1xl", bufs=1) as p1xl, \
                 tc.tile_pool(name="p1x", bufs=2) as p1x, \
                 tc.tile_pool(name="p1ps", bufs=4, space="PSUM") as p1ps:

                w_r = p1w.tile([P, KO, 3 * NH_LOCAL * P], BF, tag="wr")
                for i, wdram in enumerate((wq, wk, wv)):
                    for wc in range(4):
                        w_land = p1wl.tile([P, KO, P], F32, tag="wland")
                        nc.sync.dma_start(
                            w_land[:], wdram.ap()[:, :, wc * P:(wc + 1) * P])
                        nc.scalar.copy(
                            w_r[:, :, (4 * i + wc) * P:(4 * i + wc + 1) * P],
                            w_land[:])

                for xc in range(NXCH):
                    x_land = p1xl.tile([P, KO, XCH], F32, tag="xland")
                    nc.sync.dma_start(
                        x_land[:], xt.ap()[:, :, xc * XCH:(xc + 1) * XCH])
                    x_r = p1x.tile([P, KO, XCH], BF, tag="xr")
                    dve_war_touch(x_r)
                    nc.vector.tensor_copy(x_r[:], x_land[:])

                    for ot in range(2 * NH_LOCAL):  # Q then K o-tiles
                        ps = p1ps.tile([P, XCH], F32, tag="p1qk")
                        dummy_mm(ps)
                        for k in range(KO):
                            nc.tensor.matmul(
                                ps[:], w_r[:, k, ot * P:(ot + 1) * P],
                                x_r[:, k], start=(k == 0),
                                stop=(k == KO - 1))
                        nc.scalar.copy(
                            qk_sb[:, ot, xc * XCH:(xc + 1) * XCH], ps[:])
                    # V: out [s_tile(128), d(512)]
                    for st in range(XCH // P):
                        stg = xc * (XCH // P) + st
                        ps = p1ps.tile([P, NH_LOCAL * P], F32, tag="p1v")
                        dummy_mm(ps)
                        for k in range(KO):
                            nc.tensor.matmul(
                                ps[:], x_r[:, k, st * P:(st + 1) * P],
                                w_r[:, k, 2 * NH_LOCAL * P:3 * NH_LOCAL * P],
                                start=(k == 0), stop=(k == KO - 1))
                        nc.scalar.copy(v_sb[:, stg, :], ps[:])

            # ================= phase 2: attention ==========================
            # per (head, q_chunk): ST = K_blk @ Q^T; est = exp(ST*scale);
            # causal mask on diagonal blocks; sums += ones@est;
            # attn += V_blk^T... (lhsT=V_blk) ; normalize via rank-1 recip.
            attn_pool = ctx.enter_context(tc.tile_pool(name="attnp", bufs=1))
            attnT = attn_pool.tile([P, NH_LOCAL, S], BF)

            with tc.tile_pool(name="p2sb", bufs=1) as p2sb, \
                 tc.tile_pool(name="p2est", bufs=4) as p2est, \
                 tc.tile_pool(name="p2st", bufs=3, space="PSUM") as p2st, \
                 tc.tile_pool(name="p2at", bufs=2, space="PSUM") as p2at, \
                 tc.tile_pool(name="p2sm", bufs=2, space="PSUM") as p2sm, \
                 tc.tile_pool(name="p2rep", bufs=1, space="PSUM") as p2rep:

                for h in range(NH_LOCAL):
                    for qc in range(NQC):
                        nkb = 4 * (qc + 1)
                        qs = qc * SQ

                        at_ps = p2at.tile([P, SQ], F32, tag="atps")
                        dummy_mm(at_ps)
                        sm_ps = p2sm.tile([1, SQ], F32, tag="smps")
                        dummy_mm(sm_ps)

                        for kb in range(nkb):
                            st_ps = p2st.tile([P, SQ], F32, tag="stps")
                            nc.tensor.matmul(
                                st_ps[:],
                                qk_sb[:, NH_LOCAL + h, kb * P:(kb + 1) * P],
                                qk_sb[:, h, qs:qs + SQ],
                                start=True, stop=True)
                            est = p2est.tile([P, SQ], BF, tag="est")
                            nc.scalar.activation(est[:], st_ps[:], AF.Exp,
                                                 scale=SCALE)
                            if kb * P + P - 1 > qs:  # crosses the diagonal
                                nc.gpsimd.affine_select(
                                    est[:], est[:], [[1, SQ]],
                                    mybir.AluOpType.is_ge, 0.0,
                                    base=qs - kb * P, channel_multiplier=-1)
                            nc.tensor.matmul(sm_ps[:], ones_bf[:], est[:],
                                             start=(kb == 0),
                                             stop=(kb == nkb - 1))
                            nc.tensor.matmul(
                                at_ps[:],
                                v_sb[:, kb, h * P:(h + 1) * P], est[:],
                                start=(kb == 0), stop=(kb == nkb - 1))

                        logs = p2sb.tile([1, SQ], F32, tag="logs")
                        nc.scalar.activation(logs[:], sm_ps[:], AF.Ln)
                        recip = p2sb.tile([1, SQ], F32R, tag="recip")
                        nc.scalar.activation(recip[:], logs[:], AF.Exp,
                                             scale=-1.0)
                        rep_ps = p2rep.tile([P, SQ], F32, tag="repps")
                        dummy_mm(rep_ps)
                        nc.tensor.matmul(rep_ps[:], onesrow_r[:], recip[:],
                                         start=True, stop=True)
                        rep_sb = p2sb.tile([P, SQ], F32, tag="repsb")
                        nc.vector.tensor_copy(rep_sb[:], rep_ps[:])
                        nc.vector.tensor_mul(attnT[:, h, qs:qs + SQ],
                                             at_ps[:], rep_sb[:])

            # ================= phase 3: o_proj partial =====================
            with tc.tile_pool(name="p3wl", bufs=1) as p3wl, \
                 tc.tile_pool(name="p3w", bufs=1) as p3w, \
                 tc.tile_pool(name="p3sb", bufs=2) as p3sb, \
                 tc.tile_pool(name="p3ps", bufs=4, space="PSUM") as p3ps:
                wo_r = p3w.tile([P, NH_LOCAL, S], BF)
                for wc in range(4):
                    wo_land = p3wl.tile([P, NH_LOCAL, SQ], F32, tag="woland")
                    nc.sync.dma_start(
                        wo_land[:], wo.ap()[:, :, wc * SQ:(wc + 1) * SQ])
                    nc.scalar.copy(wo_r[:, :, wc * SQ:(wc + 1) * SQ],
                                   wo_land[:])
                # absorb {ACT wo_r} once so first real MM needs only attnT
                p3dummy = p3ps.tile([1, 16], F32, tag="p3dummy",
                                    name="p3dummy")
                nc.tensor.matmul(p3dummy[:], wo_r[:, 0, 0:1], ones16_bf[:],
                                 start=True, stop=True, skip_group_check=True)
                for ot in range(KO):
                    stage = p3sb.tile([P, S], F32, tag="p3stage")
                    act_war_touch(stage)
                    for sc in range(NQC):
                        ps = p3ps.tile([P, SQ], F32, tag="p3ps")
                        dummy_mm(ps)
                        for kb in range(NH_LOCAL):
                            nc.tensor.matmul(
                                ps[:], wo_r[:, kb, ot * P:(ot + 1) * P],
                                attnT[:, kb, sc * SQ:(sc + 1) * SQ],
                                start=(kb == 0), stop=(kb == NH_LOCAL - 1))
                        nc.scalar.copy(stage[:, sc * SQ:(sc + 1) * SQ], ps[:])
                    nc.sync.dma_start(outt.ap()[:, ot, :], stage[:])
    _split_multi_waits(nc)
    return nc


_NC_CACHE = None


def _get_nc():
    global _NC_CACHE
    if _NC_CACHE is None:
        _NC_CACHE = build()
    return _NC_CACHE


def _prep_inputs(hidden_states, w_qkv, w_o):
    """Host-side shard + pre-tile for the 8 cores."""
    hidden_states = np.asarray(hidden_states, dtype=np.float32)
    w_qkv = np.asarray(w_qkv, dtype=np.float32)
    w_o = np.asarray(w_o, dtype=np.float32)
    B = hidden_states.shape[0]

    in_maps = []
    xt_by_b = {}
    for b in range(B):
        # xt[p, ko, s] = hidden[b, s, ko*128+p]
        xt = np.ascontiguousarray(
            hidden_states[b].T.reshape(KO, P, S).transpose(1, 0, 2))
        xt_by_b[b] = xt
    for c in range(8):
        b = c // 4
        hs = [4 * (c % 4) + j for j in range(NH_LOCAL)]
        q_rows = np.concatenate([np.arange(h * P, (h + 1) * P) for h in hs])
        k_rows = q_rows + H
        v_rows = q_rows + 2 * H

        def wtile(rows):
            # [p, ko, o] = w_qkv[rows[o], ko*128+p]
            w = w_qkv[rows, :]                      # [512, 2048]
            return np.ascontiguousarray(
                w.T.reshape(KO, P, len(rows)).transpose(1, 0, 2))

        # wo[p, kb, o] = w_o[o, cols[kb*128+p]]
        wo_c = np.ascontiguousarray(
            w_o[:, q_rows].T.reshape(NH_LOCAL, P, S).transpose(1, 0, 2))
        in_maps.append({
            "xt": xt_by_b[b],
            "wq": wtile(q_rows),
            "wk": wtile(k_rows),
            "wv": wtile(v_rows),
            "wo": wo_c,
        })
    return in_maps


def run(hidden_states, w_qkv, w_o, trace=False, trace_cores=None):
    in_maps = _prep_inputs(hidden_states, w_qkv, w_o)
    nc = _get_nc()
    kwargs = {}
    if trace:
        kwargs["trace_cores"] = (trace_cores if trace_cores is not None
                                 else list(range(8)))
    res = run_bass_kernel_spmd(nc, in_maps, core_ids=list(range(8)),
                               trace=trace, **kwargs)
    B, S_, H_ = np.asarray(hidden_states).shape
    out = np.zeros((B, S_, H_), dtype=np.float32)
    for c in range(8):
        b = c // 4
        outt = res.results[c]["outt"]               # [128, 16, 2048]
        outT = outt.transpose(1, 0, 2).reshape(H_, S_)
        out[b] += outT.T
    return out, res


def kernel(hidden_states, w_qkv, w_o):
    out, _ = run(hidden_states, w_qkv, w_o, trace=False)
    return out


# revision 31
# speedup vs baseline: 1.3972x; 1.0678x over previous
"""Tensor-parallel causal attention kernel for 8 trn2 NeuronCores.

Problem: B=2, S=2048, H=2048, 16 heads, head_dim=128 fp32.
  qkv = hidden @ w_qkv.T ; causal attention ; out = attn @ w_o.T

Sharding (hardcoded): core c in 0..7 handles batch b=c//4 and heads
hs = [4*(c%4) .. 4*(c%4)+3].  Each core computes a partial o_proj
output (contraction over its 512 hidden dims); the host sums the 4
partials per batch and transposes.  No device collectives.

Device-side layout (all host-pretiled to partition-major [128, ...]):
  xt  [128,16,2048] f32 : xt[p,ko,s]  = hidden[b, s, ko*128+p]
  wq  [128,16, 512] f32 : wq[p,ko,o]  = w_qkv[q_rows[o],  ko*128+p]
  wk  [128,16, 512] f32 : wk[p,ko,o]  = w_qkv[k_rows[o],  ko*128+p]
  wv  [128,16, 512] f32 : wv[p,ko,d]  = w_qkv[v_rows[d],  ko*128+p]
  wo  [128, 4,2048] f32 : wo[p,kb,o]  = w_o[o, cols[kb*128+p]]
  outt[128,16,2048] f32 : outt[p,ot,s] = outT_partial[ot*128+p, s]

Toolchain quirks this code works around (walrus 1-sync-wait slots):
  - chunked tail drain monkeypatch
  - fp32r operands produced by compute ops (ACT/DVE), same-engine pairing
  - tiny "observer"/"dummy-matmul" ops so any instruction needs at most
    one fresh semaphore
"""
import numpy as np

import concourse.bass as bass
import concourse.mybir as mybir
import concourse.tile as tile
from concourse.bass_utils import run_bass_kernel_spmd
from concourse.vector_clock import ScopedClock, VectorClock

P = 128
S = 2048
H = 2048
NH_LOCAL = 4          # heads per core
KO = H // P           # 16 contraction chunks for the projections
SQ = 512              # q chunk width
NQC = S // SQ         # 4 q chunks
NKB = S // P          # 16 key blocks
F32 = mybir.dt.float32
F32R = mybir.dt.float32r
BF = mybir.dt.bfloat16
AF = mybir.ActivationFunctionType
SCALE = 1.0 / float(np.sqrt(128.0))

XCH = 512             # x chunk width in phase 1
NXCH = S // XCH       # 4 chunks


def _drain_and_barrier_chunked(self, tick_clock, wait_clock, _MAX=1):
    """Split the kernel-tail drain's waits: walrus allows only one sync
    wait per instruction in this toolchain."""
    g = tick_clock.global_clock
    n = len(g)
    vals = [g[i] for i in range(n)]
    nz = [i for i, v in enumerate(vals) if v > 0]
    chunks = [nz[i:i + _MAX] for i in range(0, len(nz), _MAX)] or [[]]
    for chunk in chunks:
        vec = [vals[i] if i in chunk else 0 for i in range(n)]
        d = self.nc.sync.drain()
        wait_clock.add_sem_waits(d.ins, ScopedClock({None: VectorClock(vec)}))
    self.nc.all_engine_barrier()
    assert self.sems is not None
    popped = self.nc._tile_sem_poison_stack.pop()
    assert popped is self._sem_poison
    self.nc.clear_and_free_semaphores(list(self.sems.allocated().values()))
    self.nc.all_engine_barrier()


tile.TileContext._drain_and_barrier = _drain_and_barrier_chunked


def _split_multi_waits(nc):
    """walrus allows ONE sync wait per instruction: hoist extra waits onto
    same-engine NoOps inserted directly before the offending instruction
    (identical semantics — the engine queue blocks on each in turn)."""
    ctr = 0
    for f in nc.m.functions:
        for blk in f.blocks:
            new = []
            changed = False
            for inst in blk.instructions:
                si = inst.sync_info
                waits = list(si.on_wait) if si and si.on_wait else []
                if len(waits) > 1:
                    changed = True
                    for w in waits[:-1]:
                        ctr += 1
                        nop = mybir.InstNoOp(name=f"I-wsplit-{ctr}",
                                             engine=inst.engine,
                                             ins=[], outs=[])
                        nop.sync_info = mybir.SyncInfo(on_wait=[w],
                                                       on_update=[])
                        new.append(nop)
                    ups = list(si.on_update) if si.on_update else []
                    inst.sync_info = mybir.SyncInfo(on_wait=[waits[-1]],
                                                    on_update=ups)
                new.append(inst)
            if changed:
                blk.instructions = new
    return ctr


def build():
    nc = bass.Bass()
    xt = nc.dram_tensor("xt", [P, KO, S], BF, kind="ExternalInput")
    wq = nc.dram_tensor("wq", [P, KO, NH_LOCAL * P], BF, kind="ExternalInput")
    wk = nc.dram_tensor("wk", [P, KO, NH_LOCAL * P], BF, kind="ExternalInput")
    wv = nc.dram_tensor("wv", [P, KO, NH_LOCAL * P], BF, kind="ExternalInput")
    wo = nc.dram_tensor("wo", [P, NH_LOCAL, S], BF, kind="ExternalInput")
    outt = nc.dram_tensor("outt", [P, KO, S], F32, kind="ExternalOutput")

    with tile.TileContext(nc) as tc:
        from contextlib import ExitStack
        with ExitStack() as ctx:
            const = ctx.enter_context(tc.tile_pool(name="const", bufs=1))

            # ---- constants -------------------------------------------------
            ones_f = const.tile([P, 1], F32)
            nc.vector.memset(ones_f[:], 1.0)
            ones_r = const.tile([P, 1], F32R)
            nc.scalar.copy(ones_r[:], ones_f[:])
            ones_bf = const.tile([P, 1], BF)
            nc.scalar.copy(ones_bf[:], ones_f[:])
            onesrow_f = const.tile([1, P], F32)
            nc.vector.memset(onesrow_f[:], 1.0)
            onesrow_r = const.tile([1, P], F32R)
            nc.scalar.copy(onesrow_r[:], onesrow_f[:])
            ones16_f = const.tile([P, 16], F32)
            nc.vector.memset(ones16_f[:], 1.0)
            ones16_r = const.tile([P, 16], F32R)
            nc.scalar.copy(ones16_r[:], ones16_f[:])
            ones16_bf = const.tile([P, 16], BF)
            nc.scalar.copy(ones16_bf[:], ones16_f[:])
            # observer scratch tiles (one per engine that needs them)
            obs_act = const.tile([1, 1], F32)
            nc.vector.memset(obs_act[:], 0.0)
            obs_dve = const.tile([1, 1], F32)
            nc.vector.memset(obs_dve[:], 0.0)

            def _one(ap):
                return ap[tuple(slice(0, 1) for _ in ap.shape)]

            # WAR-absorbers: a tiny write into a to-be-reused slot makes the
            # writing engine observe the slot's previous readers' semaphore,
            # so the real (full) write right after needs only its RAW sem.
            def act_war_touch(ap):
                nc.scalar.copy(_one(ap[:]), obs_act[:])

            def dve_war_touch(ap):
                nc.vector.tensor_copy(_one(ap[:]), obs_dve[:])

            def dummy_mm(ps_tile):
                # tiny PE op to absorb this psum slot's release semaphore
                nc.tensor.matmul(ps_tile[0:1, 0:16], ones_r[:], ones16_r[:],
                                 start=True, stop=True, skip_group_check=True)

            # ---- residents (phase 1+2, freed before phase 3) --------------
            qkv_pool = ctx.enter_context(tc.tile_pool(name="qkvp", bufs=1))
            # Q,K as qkvT: [d_in, o_tile(0-3 Q heads, 4-7 K heads), s]
            qk_sb = qkv_pool.tile([P, 2 * NH_LOCAL, S], BF)
            # V as [s_in, s_tile, d_local]
            v_sb = qkv_pool.tile([P, NKB, NH_LOCAL * P], BF)

            # ================= phase 1: QKV projection =====================
            # single pass over x; inputs arrive pre-cast to bf16 so DMA
            # lands directly into matmul operand tiles (no rounding pass).
            # w_r dims 0:512 = Q heads, 512:1024 = K heads, 1024:1536 = V
            with tc.tile_pool(name="p1w", bufs=1) as p1w, \
                 tc.tile_pool(name="p1x", bufs=3) as p1x, \
                 tc.tile_pool(name="p1ps", bufs=4, space="PSUM") as p1ps:

                w_r = p1w.tile([P, KO, 3 * NH_LOCAL * P], BF, tag="wr")
                for i, wdram in enumerate((wq, wk, wv)):
                    nc.sync.dma_start(
                        w_r[:, :, 4 * i * P:4 * (i + 1) * P], wdram.ap())

                for xc in range(NXCH):
                    x_r = p1x.tile([P, KO, XCH], BF, tag="xr")
                    nc.sync.dma_start(
                        x_r[:], xt.ap()[:, :, xc * XCH:(xc + 1) * XCH])

                    for ot in range(2 * NH_LOCAL):  # Q then K o-tiles
                        ps = p1ps.tile([P, XCH], F32, tag="p1qk")
                        dummy_mm(ps)
                        for k in range(KO):
                            nc.tensor.matmul(
                                ps[:], w_r[:, k, ot * P:(ot + 1) * P],
                                x_r[:, k], start=(k == 0),
                                stop=(k == KO - 1))
                        nc.scalar.copy(
                            qk_sb[:, ot, xc * XCH:(xc + 1) * XCH], ps[:])
                    # V: out [s_tile(128), d(512)]
                    for st in range(XCH // P):
                        stg = xc * (XCH // P) + st
                        ps = p1ps.tile([P, NH_LOCAL * P], F32, tag="p1v")
                        dummy_mm(ps)
                        for k in range(KO):
                            nc.tensor.matmul(
                                ps[:], x_r[:, k, st * P:(st + 1) * P],
                                w_r[:, k, 2 * NH_LOCAL * P:3 * NH_LOCAL * P],
                                start=(k == 0), stop=(k == KO - 1))
                        nc.scalar.copy(v_sb[:, stg, :], ps[:])

            # ================= phase 2: attention ==========================
            # per (head, q_chunk): ST = K_blk @ Q^T; est = exp(ST*scale);
            # causal mask on diagonal blocks; sums += ones@est;
            # attn += V_blk^T... (lhsT=V_blk) ; normalize via rank-1 recip.
            attn_pool = ctx.enter_context(tc.tile_pool(name="attnp", bufs=1))
            attnT = attn_pool.tile([P, NH_LOCAL, S], BF)

            # prefetch + pre-round w_o while attention runs
            p3w = ctx.enter_context(tc.tile_pool(name="p3w", bufs=1))
            wo_r = p3w.tile([P, NH_LOCAL, S], BF)
            nc.sync.dma_start(wo_r[:], wo.ap())

            with tc.tile_pool(name="p2sb", bufs=1) as p2sb, \
                 tc.tile_pool(name="p2est", bufs=4) as p2est, \
                 tc.tile_pool(name="p2st", bufs=2, space="PSUM") as p2st, \
                 tc.tile_pool(name="p2at", bufs=2, space="PSUM") as p2at, \
                 tc.tile_pool(name="p2sm", bufs=1, space="PSUM") as p2sm, \
                 tc.tile_pool(name="p2rep", bufs=1, space="PSUM") as p2rep:

                for h in range(NH_LOCAL):
                    for qc in range(NQC):
                        nkb = 4 * (qc + 1)
                        qs = qc * SQ

                        at_ps = p2at.tile([P, SQ], F32, tag="atps")
                        dummy_mm(at_ps)
                        sm_ps = p2sm.tile([1, SQ], F32, tag="smps")
                        dummy_mm(sm_ps)

                        # k-blocks in pairs: one [128,1024] exp per pair
                        for kp in range(nkb // 2):
                            st_ps = p2st.tile([P, 2 * SQ], F32, tag="stps")
                            est = p2est.tile([P, 2 * SQ], BF, tag="est")
                            for j in range(2):
                                kb = 2 * kp + j
                                sl = slice(j * SQ, (j + 1) * SQ)
                                nc.tensor.matmul(
                                    st_ps[:, sl],
                                    qk_sb[:, NH_LOCAL + h,
                                          kb * P:(kb + 1) * P],
                                    qk_sb[:, h, qs:qs + SQ],
                                    start=True, stop=True)
                            nc.scalar.activation(est[:], st_ps[:], AF.Exp,
                                                 scale=SCALE)
                            for j in range(2):
                                kb = 2 * kp + j
                                sl = slice(j * SQ, (j + 1) * SQ)
                                if kb * P + P - 1 > qs:  # crosses diagonal
                                    nc.gpsimd.affine_select(
                                        est[:, sl], est[:, sl], [[1, SQ]],
                                        mybir.AluOpType.is_ge, 0.0,
                                        base=qs - kb * P,
                                        channel_multiplier=-1)
                                nc.tensor.matmul(sm_ps[:], ones_bf[:],
                                                 est[:, sl],
                                                 start=(kb == 0),
                                                 stop=(kb == nkb - 1))
                                nc.tensor.matmul(
                                    at_ps[:],
                                    v_sb[:, kb, h * P:(h + 1) * P],
                                    est[:, sl],
                                    start=(kb == 0), stop=(kb == nkb - 1))

                        logs = p2sb.tile([1, SQ], F32, tag="logs")
                        nc.scalar.activation(logs[:], sm_ps[:], AF.Ln)
                        recip = p2sb.tile([1, SQ], F32R, tag="recip")
                        nc.scalar.activation(recip[:], logs[:], AF.Exp,
                                             scale=-1.0)
                        rep_ps = p2rep.tile([P, SQ], F32, tag="repps")
                        dummy_mm(rep_ps)
                        nc.tensor.matmul(rep_ps[:], onesrow_r[:], recip[:],
                                         start=True, stop=True)
                        rep_sb = p2sb.tile([P, SQ], F32, tag="repsb")
                        nc.vector.tensor_copy(rep_sb[:], rep_ps[:])
                        nc.vector.tensor_mul(attnT[:, h, qs:qs + SQ],
                                             at_ps[:], rep_sb[:])

            # ================= phase 3: o_proj partial =====================
            with tc.tile_pool(name="p3sb", bufs=2) as p3sb, \
                 tc.tile_pool(name="p3ps", bufs=4, space="PSUM") as p3ps:
                for ot in range(KO):
                    stage = p3sb.tile([P, S], F32, tag="p3stage")
                    act_war_touch(stage)
                    for sc in range(NQC):
                        ps = p3ps.tile([P, SQ], F32, tag="p3ps")
                        dummy_mm(ps)
                        for kb in range(NH_LOCAL):
                            nc.tensor.matmul(
                                ps[:], wo_r[:, kb, ot * P:(ot + 1) * P],
                                attnT[:, kb, sc * SQ:(sc + 1) * SQ],
                                start=(kb == 0), stop=(kb == NH_LOCAL - 1))
                        nc.scalar.copy(stage[:, sc * SQ:(sc + 1) * SQ], ps[:])
                    nc.sync.dma_start(outt.ap()[:, ot, :], stage[:])
    _split_multi_waits(nc)
    return nc


_NC_CACHE = None


def _get_nc():
    global _NC_CACHE
    if _NC_CACHE is None:
        _NC_CACHE = build()
    return _NC_CACHE


def _prep_inputs(hidden_states, w_qkv, w_o):
    """Host-side shard + pre-tile + bf16-cast for the 8 cores."""
    import ml_dtypes
    BF_NP = ml_dtypes.bfloat16
    hidden_states = np.asarray(hidden_states, dtype=np.float32)
    w_qkv = np.asarray(w_qkv, dtype=np.float32)
    w_o = np.asarray(w_o, dtype=np.float32)
    B = hidden_states.shape[0]

    in_maps = []
    xt_by_b = {}
    for b in range(B):
        # xt[p, ko, s] = hidden[b, s, ko*128+p]
        xt = np.ascontiguousarray(
            hidden_states[b].T.reshape(KO, P, S).transpose(1, 0, 2)
        ).astype(BF_NP)
        xt_by_b[b] = xt
    for c in range(8):
        b = c // 4
        hs = [4 * (c % 4) + j for j in range(NH_LOCAL)]
        q_rows = np.concatenate([np.arange(h * P, (h + 1) * P) for h in hs])
        k_rows = q_rows + H
        v_rows = q_rows + 2 * H

        def wtile(rows):
            # [p, ko, o] = w_qkv[rows[o], ko*128+p]
            w = w_qkv[rows, :]                      # [512, 2048]
            return np.ascontiguousarray(
                w.T.reshape(KO, P, len(rows)).transpose(1, 0, 2)
            ).astype(BF_NP)

        # wo[p, kb, o] = w_o[o, cols[kb*128+p]]
        wo_c = np.ascontiguousarray(
            w_o[:, q_rows].T.reshape(NH_LOCAL, P, S).transpose(1, 0, 2)
        ).astype(BF_NP)
        in_maps.append({
            "xt": xt_by_b[b],
            "wq": wtile(q_rows),
            "wk": wtile(k_rows),
            "wv": wtile(v_rows),
            "wo": wo_c,
        })
    return in_maps


def run(hidden_states, w_qkv, w_o, trace=False, trace_cores=None):
    in_maps = _prep_inputs(hidden_states, w_qkv, w_o)
    nc = _get_nc()
    kwargs = {}
    if trace:
        kwargs["trace_cores"] = (trace_cores if trace_cores is not None
                                 else list(range(8)))
    res = run_bass_kernel_spmd(nc, in_maps, core_ids=list(range(8)),
                               trace=trace, **kwargs)
    B, S_, H_ = np.asarray(hidden_states).shape
    out = np.zeros((B, S_, H_), dtype=np.float32)
    for c in range(8):
        b = c // 4
        outt = res.results[c]["outt"]               # [128, 16, 2048]
        outT = outt.transpose(1, 0, 2).reshape(H_, S_)
        out[b] += outT.T
    return out, res


def kernel(hidden_states, w_qkv, w_o):
    out, _ = run(hidden_states, w_qkv, w_o, trace=False)
    return out


# revision 37
# speedup vs baseline: 1.4480x; 1.0363x over previous
"""Tensor-parallel causal attention kernel for 8 trn2 NeuronCores.

Problem: B=2, S=2048, H=2048, 16 heads, head_dim=128 fp32.
  qkv = hidden @ w_qkv.T ; causal attention ; out = attn @ w_o.T

Sharding (hardcoded): core c in 0..7 handles batch b=c//4 and heads
hs = [4*(c%4) .. 4*(c%4)+3].  Each core computes a partial o_proj
output (contraction over its 512 hidden dims); the host sums the 4
partials per batch and transposes.  No device collectives.

Device-side layout (all host-pretiled to partition-major [128, ...]):
  xt  [128,16,2048] f32 : xt[p,ko,s]  = hidden[b, s, ko*128+p]
  wq  [128,16, 512] f32 : wq[p,ko,o]  = w_qkv[q_rows[o],  ko*128+p]
  wk  [128,16, 512] f32 : wk[p,ko,o]  = w_qkv[k_rows[o],  ko*128+p]
  wv  [128,16, 512] f32 : wv[p,ko,d]  = w_qkv[v_rows[d],  ko*128+p]
  wo  [128, 4,2048] f32 : wo[p,kb,o]  = w_o[o, cols[kb*128+p]]
  outt[128,16,2048] f32 : outt[p,ot,s] = outT_partial[ot*128+p, s]

Toolchain quirks this code works around (walrus 1-sync-wait slots):
  - chunked tail drain monkeypatch
  - fp32r operands produced by compute ops (ACT/DVE), same-engine pairing
  - tiny "observer"/"dummy-matmul" ops so any instruction needs at most
    one fresh semaphore
"""
import numpy as np

import concourse.bass as bass
import concourse.mybir as mybir
import concourse.tile as tile
from concourse.bass_utils import run_bass_kernel_spmd
from concourse.vector_clock import ScopedClock, VectorClock

P = 128
S = 2048
H = 2048
NH_LOCAL = 4          # heads per core
KO = H // P           # 16 contraction chunks for the projections
SQ = 512              # q chunk width
NQC = S // SQ         # 4 q chunks
NKB = S // P          # 16 key blocks
F32 = mybir.dt.float32
F32R = mybir.dt.float32r
BF = mybir.dt.bfloat16
AF = mybir.ActivationFunctionType
SCALE = 1.0 / float(np.sqrt(128.0))

XCH = 512             # x chunk width in phase 1
NXCH = S // XCH       # 4 chunks


def _drain_and_barrier_chunked(self, tick_clock, wait_clock, _MAX=1):
    """Split the kernel-tail drain's waits: walrus allows only one sync
    wait per instruction in this toolchain."""
    g = tick_clock.global_clock
    n = len(g)
    vals = [g[i] for i in range(n)]
    nz = [i for i, v in enumerate(vals) if v > 0]
    chunks = [nz[i:i + _MAX] for i in range(0, len(nz), _MAX)] or [[]]
    for chunk in chunks:
        vec = [vals[i] if i in chunk else 0 for i in range(n)]
        d = self.nc.sync.drain()
        wait_clock.add_sem_waits(d.ins, ScopedClock({None: VectorClock(vec)}))
    self.nc.all_engine_barrier()
    assert self.sems is not None
    popped = self.nc._tile_sem_poison_stack.pop()
    assert popped is self._sem_poison
    self.nc.clear_and_free_semaphores(list(self.sems.allocated().values()))
    self.nc.all_engine_barrier()


tile.TileContext._drain_and_barrier = _drain_and_barrier_chunked


def _split_multi_waits(nc):
    """walrus allows ONE sync wait per instruction: hoist extra waits onto
    same-engine NoOps inserted directly before the offending instruction
    (identical semantics — the engine queue blocks on each in turn)."""
    ctr = 0
    for f in nc.m.functions:
        for blk in f.blocks:
            new = []
            changed = False
            for inst in blk.instructions:
                si = inst.sync_info
                waits = list(si.on_wait) if si and si.on_wait else []
                if len(waits) > 1:
                    changed = True
                    for w in waits[:-1]:
                        ctr += 1
                        nop = mybir.InstNoOp(name=f"I-wsplit-{ctr}",
                                             engine=inst.engine,
                                             ins=[], outs=[])
                        nop.sync_info = mybir.SyncInfo(on_wait=[w],
                                                       on_update=[])
                        new.append(nop)
                    ups = list(si.on_update) if si.on_update else []
                    inst.sync_info = mybir.SyncInfo(on_wait=[waits[-1]],
                                                    on_update=ups)
                new.append(inst)
            if changed:
                blk.instructions = new
    return ctr


def build():
    nc = bass.Bass()
    xt = nc.dram_tensor("xt", [P, KO, S], BF, kind="ExternalInput")
    wq = nc.dram_tensor("wq", [P, KO, NH_LOCAL * P], BF, kind="ExternalInput")
    wk = nc.dram_tensor("wk", [P, KO, NH_LOCAL * P], BF, kind="ExternalInput")
    wv = nc.dram_tensor("wv", [P, KO, NH_LOCAL * P], BF, kind="ExternalInput")
    wo = nc.dram_tensor("wo", [P, NH_LOCAL, S], BF, kind="ExternalInput")
    outt = nc.dram_tensor("outt", [P, KO, S], F32, kind="ExternalOutput")

    with tile.TileContext(nc) as tc:
        from contextlib import ExitStack
        with ExitStack() as ctx:
            const = ctx.enter_context(tc.tile_pool(name="const", bufs=1))

            # ---- constants -------------------------------------------------
            ones_f = const.tile([P, 1], F32)
            nc.vector.memset(ones_f[:], 1.0)
            ones_r = const.tile([P, 1], F32R)
            nc.scalar.copy(ones_r[:], ones_f[:])
            ones_bf = const.tile([P, 1], BF)
            nc.scalar.copy(ones_bf[:], ones_f[:])
            onesrow_f = const.tile([1, P], F32)
            nc.vector.memset(onesrow_f[:], 1.0)
            onesrow_r = const.tile([1, P], F32R)
            nc.scalar.copy(onesrow_r[:], onesrow_f[:])
            ones16_f = const.tile([P, 16], F32)
            nc.vector.memset(ones16_f[:], 1.0)
            ones16_r = const.tile([P, 16], F32R)
            nc.scalar.copy(ones16_r[:], ones16_f[:])
            ones16_bf = const.tile([P, 16], BF)
            nc.scalar.copy(ones16_bf[:], ones16_f[:])
            # observer scratch tiles (one per engine that needs them)
            obs_act = const.tile([1, 1], F32)
            nc.vector.memset(obs_act[:], 0.0)
            obs_dve = const.tile([1, 1], F32)
            nc.vector.memset(obs_dve[:], 0.0)

            def _one(ap):
                return ap[tuple(slice(0, 1) for _ in ap.shape)]

            # WAR-absorbers: a tiny write into a to-be-reused slot makes the
            # writing engine observe the slot's previous readers' semaphore,
            # so the real (full) write right after needs only its RAW sem.
            def act_war_touch(ap):
                nc.scalar.copy(_one(ap[:]), obs_act[:])

            def dve_war_touch(ap):
                nc.vector.tensor_copy(_one(ap[:]), obs_dve[:])

            def dummy_mm(ps_tile):
                # tiny PE op to absorb this psum slot's release semaphore
                nc.tensor.matmul(ps_tile[0:1, 0:16], ones_r[:], ones16_r[:],
                                 start=True, stop=True, skip_group_check=True)

            # ---- residents (phase 1+2, freed before phase 3) --------------
            qkv_pool = ctx.enter_context(tc.tile_pool(name="qkvp", bufs=1))
            # Q,K as qkvT: [d_in, o_tile(0-3 Q heads, 4-7 K heads), s]
            qk_sb = qkv_pool.tile([P, 2 * NH_LOCAL, S], BF)
            # V as [s_in, s_tile, d_local]
            v_sb = qkv_pool.tile([P, NKB, NH_LOCAL * P], BF)

            # ================= phase 1: QKV projection =====================
            # single pass over x; inputs arrive pre-cast to bf16 so DMA
            # lands directly into matmul operand tiles (no rounding pass).
            # w_r dims 0:512 = Q heads, 512:1024 = K heads, 1024:1536 = V
            with tc.tile_pool(name="p1w", bufs=1) as p1w, \
                 tc.tile_pool(name="p1x", bufs=3) as p1x, \
                 tc.tile_pool(name="p1ps", bufs=4, space="PSUM") as p1ps:

                # x chunk 0 + wq first so the first matmuls start ASAP
                w_r = p1w.tile([P, KO, 3 * NH_LOCAL * P], BF, tag="wr")
                x_tiles = []
                x_r0 = p1x.tile([P, KO, XCH], BF, tag="xr", name="xr0")
                nc.sync.dma_start(x_r0[:], xt.ap()[:, :, 0:XCH])
                x_tiles.append(x_r0)
                for i, wdram in enumerate((wq, wk, wv)):
                    nc.sync.dma_start(
                        w_r[:, :, 4 * i * P:4 * (i + 1) * P], wdram.ap())

                for xc in range(NXCH):
                    if xc > 0:
                        x_r = p1x.tile([P, KO, XCH], BF, tag="xr")
                        nc.sync.dma_start(
                            x_r[:], xt.ap()[:, :, xc * XCH:(xc + 1) * XCH])
                    else:
                        x_r = x_tiles[0]

                    for ot in range(2 * NH_LOCAL):  # Q then K o-tiles
                        ps = p1ps.tile([P, XCH], F32, tag="p1qk")
                        dummy_mm(ps)
                        for k in range(KO):
                            nc.tensor.matmul(
                                ps[:], w_r[:, k, ot * P:(ot + 1) * P],
                                x_r[:, k], start=(k == 0),
                                stop=(k == KO - 1))
                        nc.vector.tensor_copy(
                            qk_sb[:, ot, xc * XCH:(xc + 1) * XCH], ps[:])
                    # V: out [s_tile(128), d(512)]
                    for st in range(XCH // P):
                        stg = xc * (XCH // P) + st
                        ps = p1ps.tile([P, NH_LOCAL * P], F32, tag="p1v")
                        dummy_mm(ps)
                        for k in range(KO):
                            nc.tensor.matmul(
                                ps[:], x_r[:, k, st * P:(st + 1) * P],
                                w_r[:, k, 2 * NH_LOCAL * P:3 * NH_LOCAL * P],
                                start=(k == 0), stop=(k == KO - 1))
                        nc.vector.tensor_copy(v_sb[:, stg, :], ps[:])

            # ================= phase 2: attention ==========================
            # per (head, q_chunk): ST = K_blk @ Q^T; est = exp(ST*scale);
            # causal mask on diagonal blocks; sums += ones@est;
            # attn += V_blk^T... (lhsT=V_blk) ; normalize via rank-1 recip.
            attn_pool = ctx.enter_context(tc.tile_pool(name="attnp", bufs=1))
            attnT = attn_pool.tile([P, NH_LOCAL, S], BF)

            # prefetch + pre-round w_o while attention runs
            p3w = ctx.enter_context(tc.tile_pool(name="p3w", bufs=1))
            wo_r = p3w.tile([P, NH_LOCAL, S], BF)
            nc.sync.dma_start(wo_r[:], wo.ap())

            with tc.tile_pool(name="p2sb", bufs=1) as p2sb, \
                 tc.tile_pool(name="p2est", bufs=6) as p2est, \
                 tc.tile_pool(name="p2st", bufs=2, space="PSUM") as p2st, \
                 tc.tile_pool(name="p2at", bufs=2, space="PSUM") as p2at, \
                 tc.tile_pool(name="p2sm", bufs=1, space="PSUM") as p2sm, \
                 tc.tile_pool(name="p2rep", bufs=1, space="PSUM") as p2rep:

                for h in range(NH_LOCAL):
                    for qc in range(NQC):
                        nkb = 4 * (qc + 1)
                        qs = qc * SQ

                        at_ps = p2at.tile([P, SQ], F32, tag="atps")
                        dummy_mm(at_ps)
                        sm_ps = p2sm.tile([1, SQ], F32, tag="smps")
                        dummy_mm(sm_ps)

                        # k-blocks in pairs: one [128,1024] exp per pair
                        for kp in range(nkb // 2):
                            st_ps = p2st.tile([P, 2 * SQ], F32, tag="stps")
                            est = p2est.tile([P, 2 * SQ], BF, tag="est")
                            for j in range(2):
                                kb = 2 * kp + j
                                sl = slice(j * SQ, (j + 1) * SQ)
                                nc.tensor.matmul(
                                    st_ps[:, sl],
                                    qk_sb[:, NH_LOCAL + h,
                                          kb * P:(kb + 1) * P],
                                    qk_sb[:, h, qs:qs + SQ],
                                    start=True, stop=True)
                            nc.scalar.activation(est[:], st_ps[:], AF.Exp,
                                                 scale=SCALE)
                            for j in range(2):
                                kb = 2 * kp + j
                                sl = slice(j * SQ, (j + 1) * SQ)
                                if kb * P + P - 1 > qs:  # crosses diagonal
                                    nc.gpsimd.affine_select(
                                        est[:, sl], est[:, sl], [[1, SQ]],
                                        mybir.AluOpType.is_ge, 0.0,
                                        base=qs - kb * P,
                                        channel_multiplier=-1)
                                nc.tensor.matmul(sm_ps[:], ones_bf[:],
                                                 est[:, sl],
                                                 start=(kb == 0),
                                                 stop=(kb == nkb - 1))
                                nc.tensor.matmul(
                                    at_ps[:],
                                    v_sb[:, kb, h * P:(h + 1) * P],
                                    est[:, sl],
                                    start=(kb == 0), stop=(kb == nkb - 1))

                        logs = p2sb.tile([1, SQ], F32, tag="logs")
                        nc.scalar.activation(logs[:], sm_ps[:], AF.Ln)
                        recip = p2sb.tile([1, SQ], F32R, tag="recip")
                        nc.scalar.activation(recip[:], logs[:], AF.Exp,
                                             scale=-1.0)
                        rep_ps = p2rep.tile([P, SQ], F32, tag="repps")
                        dummy_mm(rep_ps)
                        nc.tensor.matmul(rep_ps[:], onesrow_r[:], recip[:],
                                         start=True, stop=True)
                        rep_sb = p2sb.tile([P, SQ], F32, tag="repsb")
                        nc.vector.tensor_copy(rep_sb[:], rep_ps[:])
                        nc.vector.tensor_mul(attnT[:, h, qs:qs + SQ],
                                             at_ps[:], rep_sb[:])

            # ================= phase 3: o_proj partial =====================
            with tc.tile_pool(name="p3sb", bufs=2) as p3sb, \
                 tc.tile_pool(name="p3ps", bufs=4, space="PSUM") as p3ps:
                for ot in range(KO):
                    stage = p3sb.tile([P, S], F32, tag="p3stage")
                    dve_war_touch(stage)
                    for sc in range(NQC):
                        ps = p3ps.tile([P, SQ], F32, tag="p3ps")
                        dummy_mm(ps)
                        for kb in range(NH_LOCAL):
                            nc.tensor.matmul(
                                ps[:], wo_r[:, kb, ot * P:(ot + 1) * P],
                                attnT[:, kb, sc * SQ:(sc + 1) * SQ],
                                start=(kb == 0), stop=(kb == NH_LOCAL - 1))
                        nc.vector.tensor_copy(stage[:, sc * SQ:(sc + 1) * SQ],
                                              ps[:])
                    nc.sync.dma_start(outt.ap()[:, ot, :], stage[:])
    _split_multi_waits(nc)
    return nc


_NC_CACHE = None


def _get_nc():
    global _NC_CACHE
    if _NC_CACHE is None:
        _NC_CACHE = build()
    return _NC_CACHE


def _prep_inputs(hidden_states, w_qkv, w_o):
    """Host-side shard + pre-tile + bf16-cast for the 8 cores."""
    import ml_dtypes
    BF_NP = ml_dtypes.bfloat16
    hidden_states = np.asarray(hidden_states, dtype=np.float32)
    w_qkv = np.asarray(w_qkv, dtype=np.float32)
    w_o = np.asarray(w_o, dtype=np.float32)
    B = hidden_states.shape[0]

    in_maps = []
    xt_by_b = {}
    for b in range(B):
        # xt[p, ko, s] = hidden[b, s, ko*128+p]
        xt = np.ascontiguousarray(
            hidden_states[b].T.reshape(KO, P, S).transpose(1, 0, 2)
        ).astype(BF_NP)
        xt_by_b[b] = xt
    for c in range(8):
        b = c // 4
        hs = [4 * (c % 4) + j for j in range(NH_LOCAL)]
        q_rows = np.concatenate([np.arange(h * P, (h + 1) * P) for h in hs])
        k_rows = q_rows + H
        v_rows = q_rows + 2 * H

        def wtile(rows):
            # [p, ko, o] = w_qkv[rows[o], ko*128+p]
            w = w_qkv[rows, :]                      # [512, 2048]
            return np.ascontiguousarray(
                w.T.reshape(KO, P, len(rows)).transpose(1, 0, 2)
            ).astype(BF_NP)

        # wo[p, kb, o] = w_o[o, cols[kb*128+p]]
        wo_c = np.ascontiguousarray(
            w_o[:, q_rows].T.reshape(NH_LOCAL, P, S).transpose(1, 0, 2)
        ).astype(BF_NP)
        in_maps.append({
            "xt": xt_by_b[b],
            "wq": wtile(q_rows),
            "wk": wtile(k_rows),
            "wv": wtile(v_rows),
            "wo": wo_c,
        })
    return in_maps


def run(hidden_states, w_qkv, w_o, trace=False, trace_cores=None):
    in_maps = _prep_inputs(hidden_states, w_qkv, w_o)
    nc = _get_nc()
    kwargs = {}
    if trace:
        kwargs["trace_cores"] = (trace_cores if trace_cores is not None
                                 else list(range(8)))
    res = run_bass_kernel_spmd(nc, in_maps, core_ids=list(range(8)),
                               trace=trace, **kwargs)
    B, S_, H_ = np.asarray(hidden_states).shape
    out = np.zeros((B, S_, H_), dtype=np.float32)
    for c in range(8):
        b = c // 4
        outt = res.results[c]["outt"]               # [128, 16, 2048]
        outT = outt.transpose(1, 0, 2).reshape(H_, S_)
        out[b] += outT.T
    return out, res


def kernel(hidden_states, w_qkv, w_o):
    out, _ = run(hidden_states, w_qkv, w_o, trace=False)
    return out


# revision 39
# speedup vs baseline: 1.4898x; 1.0289x over previous
"""Tensor-parallel causal attention kernel for 8 trn2 NeuronCores.

Problem: B=2, S=2048, H=2048, 16 heads, head_dim=128 fp32.
  qkv = hidden @ w_qkv.T ; causal attention ; out = attn @ w_o.T

Sharding (hardcoded): core c in 0..7 handles batch b=c//4 and heads
hs = [4*(c%4) .. 4*(c%4)+3].  Each core computes a partial o_proj
output (contraction over its 512 hidden dims); the host sums the 4
partials per batch and transposes.  No device collectives.

Device-side layout (all host-pretiled to partition-major [128, ...]):
  xt  [128,16,2048] f32 : xt[p,ko,s]  = hidden[b, s, ko*128+p]
  wq  [128,16, 512] f32 : wq[p,ko,o]  = w_qkv[q_rows[o],  ko*128+p]
  wk  [128,16, 512] f32 : wk[p,ko,o]  = w_qkv[k_rows[o],  ko*128+p]
  wv  [128,16, 512] f32 : wv[p,ko,d]  = w_qkv[v_rows[d],  ko*128+p]
  wo  [128, 4,2048] f32 : wo[p,kb,o]  = w_o[o, cols[kb*128+p]]
  outt[128,16,2048] f32 : outt[p,ot,s] = outT_partial[ot*128+p, s]

Toolchain quirks this code works around (walrus 1-sync-wait slots):
  - chunked tail drain monkeypatch
  - fp32r operands produced by compute ops (ACT/DVE), same-engine pairing
  - tiny "observer"/"dummy-matmul" ops so any instruction needs at most
    one fresh semaphore
"""
import numpy as np

import concourse.bass as bass
import concourse.mybir as mybir
import concourse.tile as tile
from concourse.bass_utils import run_bass_kernel_spmd
from concourse.vector_clock import ScopedClock, VectorClock

P = 128
S = 2048
H = 2048
NH_LOCAL = 4          # heads per core
KO = H // P           # 16 contraction chunks for the projections
SQ = 512              # q chunk width
NQC = S // SQ         # 4 q chunks
NKB = S // P          # 16 key blocks
F32 = mybir.dt.float32
F32R = mybir.dt.float32r
BF = mybir.dt.bfloat16
AF = mybir.ActivationFunctionType
SCALE = 1.0 / float(np.sqrt(128.0))

XCH = 512             # x chunk width in phase 1
NXCH = S // XCH       # 4 chunks


def _drain_and_barrier_chunked(self, tick_clock, wait_clock, _MAX=1):
    """Split the kernel-tail drain's waits: walrus allows only one sync
    wait per instruction in this toolchain."""
    g = tick_clock.global_clock
    n = len(g)
    vals = [g[i] for i in range(n)]
    nz = [i for i, v in enumerate(vals) if v > 0]
    chunks = [nz[i:i + _MAX] for i in range(0, len(nz), _MAX)] or [[]]
    for chunk in chunks:
        vec = [vals[i] if i in chunk else 0 for i in range(n)]
        d = self.nc.sync.drain()
        wait_clock.add_sem_waits(d.ins, ScopedClock({None: VectorClock(vec)}))
    self.nc.all_engine_barrier()
    assert self.sems is not None
    popped = self.nc._tile_sem_poison_stack.pop()
    assert popped is self._sem_poison
    self.nc.clear_and_free_semaphores(list(self.sems.allocated().values()))
    self.nc.all_engine_barrier()


tile.TileContext._drain_and_barrier = _drain_and_barrier_chunked


def _split_multi_waits(nc):
    """walrus allows ONE sync wait per instruction: hoist extra waits onto
    same-engine NoOps inserted directly before the offending instruction
    (identical semantics — the engine queue blocks on each in turn)."""
    ctr = 0
    for f in nc.m.functions:
        for blk in f.blocks:
            new = []
            changed = False
            for inst in blk.instructions:
                si = inst.sync_info
                waits = list(si.on_wait) if si and si.on_wait else []
                if len(waits) > 1:
                    changed = True
                    for w in waits[:-1]:
                        ctr += 1
                        nop = mybir.InstNoOp(name=f"I-wsplit-{ctr}",
                                             engine=inst.engine,
                                             ins=[], outs=[])
                        nop.sync_info = mybir.SyncInfo(on_wait=[w],
                                                       on_update=[])
                        new.append(nop)
                    ups = list(si.on_update) if si.on_update else []
                    inst.sync_info = mybir.SyncInfo(on_wait=[waits[-1]],
                                                    on_update=ups)
                new.append(inst)
            if changed:
                blk.instructions = new
    return ctr


def build():
    nc = bass.Bass()
    xt = nc.dram_tensor("xt", [P, KO, S], BF, kind="ExternalInput")
    wq = nc.dram_tensor("wq", [P, KO, NH_LOCAL * P], BF, kind="ExternalInput")
    wk = nc.dram_tensor("wk", [P, KO, NH_LOCAL * P], BF, kind="ExternalInput")
    wv = nc.dram_tensor("wv", [P, KO, NH_LOCAL * P], BF, kind="ExternalInput")
    wo = nc.dram_tensor("wo", [P, NH_LOCAL, S], BF, kind="ExternalInput")
    outt = nc.dram_tensor("outt", [P, KO, S], F32, kind="ExternalOutput")

    with tile.TileContext(nc) as tc:
        from contextlib import ExitStack
        with ExitStack() as ctx:
            const = ctx.enter_context(tc.tile_pool(name="const", bufs=1))

            # ---- constants -------------------------------------------------
            ones_f = const.tile([P, 1], F32)
            nc.vector.memset(ones_f[:], 1.0)
            ones_r = const.tile([P, 1], F32R)
            nc.scalar.copy(ones_r[:], ones_f[:])
            ones_bf = const.tile([P, 1], BF)
            nc.scalar.copy(ones_bf[:], ones_f[:])
            onesrow_f = const.tile([1, P], F32)
            nc.vector.memset(onesrow_f[:], 1.0)
            onesrow_r = const.tile([1, P], F32R)
            nc.scalar.copy(onesrow_r[:], onesrow_f[:])
            ones16_f = const.tile([P, 16], F32)
            nc.vector.memset(ones16_f[:], 1.0)
            ones16_r = const.tile([P, 16], F32R)
            nc.scalar.copy(ones16_r[:], ones16_f[:])
            ones16_bf = const.tile([P, 16], BF)
            nc.scalar.copy(ones16_bf[:], ones16_f[:])
            # observer scratch tiles (one per engine that needs them)
            obs_act = const.tile([1, 1], F32)
            nc.vector.memset(obs_act[:], 0.0)
            obs_dve = const.tile([1, 1], F32)
            nc.vector.memset(obs_dve[:], 0.0)

            def _one(ap):
                return ap[tuple(slice(0, 1) for _ in ap.shape)]

            # WAR-absorbers: a tiny write into a to-be-reused slot makes the
            # writing engine observe the slot's previous readers' semaphore,
            # so the real (full) write right after needs only its RAW sem.
            def act_war_touch(ap):
                nc.scalar.copy(_one(ap[:]), obs_act[:])

            def dve_war_touch(ap):
                nc.vector.tensor_copy(_one(ap[:]), obs_dve[:])

            def dummy_mm(ps_tile):
                # tiny PE op to absorb this psum slot's release semaphore
                nc.tensor.matmul(ps_tile[0:1, 0:16], ones_r[:], ones16_r[:],
                                 start=True, stop=True, skip_group_check=True)

            # ---- residents (phase 1+2, freed before phase 3) --------------
            qkv_pool = ctx.enter_context(tc.tile_pool(name="qkvp", bufs=1))
            # Q,K as qkvT: [d_in, o_tile(0-3 Q heads, 4-7 K heads), s]
            qk_sb = qkv_pool.tile([P, 2 * NH_LOCAL, S], BF)
            # V as [s_in, s_tile, d_local]
            v_sb = qkv_pool.tile([P, NKB, NH_LOCAL * P], BF)

            # ================= phase 1: QKV projection =====================
            # single pass over x; inputs arrive pre-cast to bf16 so DMA
            # lands directly into matmul operand tiles (no rounding pass).
            # w_r dims 0:512 = Q heads, 512:1024 = K heads, 1024:1536 = V
            with tc.tile_pool(name="p1w", bufs=1) as p1w, \
                 tc.tile_pool(name="p1x", bufs=3) as p1x, \
                 tc.tile_pool(name="p1ps", bufs=4, space="PSUM") as p1ps:

                # x chunk 0 + wq first, in ko-quarters, so the first
                # accumulation group can start after ~1/4 of the loads
                w_r = p1w.tile([P, KO, 3 * NH_LOCAL * P], BF, tag="wr")
                x_tiles = []
                x_r0 = p1x.tile([P, KO, XCH], BF, tag="xr", name="xr0")
                for kq in range(4):
                    ks = slice(4 * kq, 4 * (kq + 1))
                    nc.sync.dma_start(x_r0[:, ks], xt.ap()[:, ks, 0:XCH])
                    nc.sync.dma_start(w_r[:, ks, 0:4 * P], wq.ap()[:, ks])
                x_tiles.append(x_r0)
                for i, wdram in ((1, wk), (2, wv)):
                    nc.sync.dma_start(
                        w_r[:, :, 4 * i * P:4 * (i + 1) * P], wdram.ap())

                for xc in range(NXCH):
                    if xc > 0:
                        x_r = p1x.tile([P, KO, XCH], BF, tag="xr")
                        nc.sync.dma_start(
                            x_r[:], xt.ap()[:, :, xc * XCH:(xc + 1) * XCH])
                    else:
                        x_r = x_tiles[0]

                    for ot in range(2 * NH_LOCAL):  # Q then K o-tiles
                        ps = p1ps.tile([P, XCH], F32, tag="p1qk")
                        dummy_mm(ps)
                        for k in range(KO):
                            nc.tensor.matmul(
                                ps[:], w_r[:, k, ot * P:(ot + 1) * P],
                                x_r[:, k], start=(k == 0),
                                stop=(k == KO - 1))
                        nc.vector.tensor_copy(
                            qk_sb[:, ot, xc * XCH:(xc + 1) * XCH], ps[:])
                    # V: out [s_tile(128), d(512)]
                    for st in range(XCH // P):
                        stg = xc * (XCH // P) + st
                        ps = p1ps.tile([P, NH_LOCAL * P], F32, tag="p1v")
                        dummy_mm(ps)
                        for k in range(KO):
                            nc.tensor.matmul(
                                ps[:], x_r[:, k, st * P:(st + 1) * P],
                                w_r[:, k, 2 * NH_LOCAL * P:3 * NH_LOCAL * P],
                                start=(k == 0), stop=(k == KO - 1))
                        nc.vector.tensor_copy(v_sb[:, stg, :], ps[:])

            # ================= phase 2: attention ==========================
            # per (head, q_chunk): ST = K_blk @ Q^T; est = exp(ST*scale);
            # causal mask on diagonal blocks; sums += ones@est;
            # attn += V_blk^T... (lhsT=V_blk) ; normalize via rank-1 recip.
            attn_pool = ctx.enter_context(tc.tile_pool(name="attnp", bufs=1))
            attnT = attn_pool.tile([P, NH_LOCAL, S], BF)

            # prefetch + pre-round w_o while attention runs
            p3w = ctx.enter_context(tc.tile_pool(name="p3w", bufs=1))
            wo_r = p3w.tile([P, NH_LOCAL, S], BF)
            nc.sync.dma_start(wo_r[:], wo.ap())

            with tc.tile_pool(name="p2sb", bufs=1) as p2sb, \
                 tc.tile_pool(name="p2est", bufs=6) as p2est, \
                 tc.tile_pool(name="p2st", bufs=2, space="PSUM") as p2st, \
                 tc.tile_pool(name="p2at", bufs=2, space="PSUM") as p2at, \
                 tc.tile_pool(name="p2sm", bufs=1, space="PSUM") as p2sm, \
                 tc.tile_pool(name="p2rep", bufs=1, space="PSUM") as p2rep:

                for h in range(NH_LOCAL):
                    for qc in range(NQC):
                        nkb = 4 * (qc + 1)
                        qs = qc * SQ

                        at_ps = p2at.tile([P, SQ], F32, tag="atps")
                        dummy_mm(at_ps)
                        sm_ps = p2sm.tile([1, SQ], F32, tag="smps")
                        dummy_mm(sm_ps)

                        # k-blocks in pairs: one [128,1024] exp per pair
                        for kp in range(nkb // 2):
                            st_ps = p2st.tile([P, 2 * SQ], F32, tag="stps")
                            est = p2est.tile([P, 2 * SQ], BF, tag="est")
                            for j in range(2):
                                kb = 2 * kp + j
                                sl = slice(j * SQ, (j + 1) * SQ)
                                nc.tensor.matmul(
                                    st_ps[:, sl],
                                    qk_sb[:, NH_LOCAL + h,
                                          kb * P:(kb + 1) * P],
                                    qk_sb[:, h, qs:qs + SQ],
                                    start=True, stop=True)
                            nc.scalar.activation(est[:], st_ps[:], AF.Exp,
                                                 scale=SCALE)
                            for j in range(2):
                                kb = 2 * kp + j
                                sl = slice(j * SQ, (j + 1) * SQ)
                                if kb * P + P - 1 > qs:  # crosses diagonal
                                    nc.gpsimd.affine_select(
                                        est[:, sl], est[:, sl], [[1, SQ]],
                                        mybir.AluOpType.is_ge, 0.0,
                                        base=qs - kb * P,
                                        channel_multiplier=-1)
                                nc.tensor.matmul(sm_ps[:], ones_bf[:],
                                                 est[:, sl],
                                                 start=(kb == 0),
                                                 stop=(kb == nkb - 1))
                                nc.tensor.matmul(
                                    at_ps[:],
                                    v_sb[:, kb, h * P:(h + 1) * P],
                                    est[:, sl],
                                    start=(kb == 0), stop=(kb == nkb - 1))

                        logs = p2sb.tile([1, SQ], F32, tag="logs")
                        nc.scalar.activation(logs[:], sm_ps[:], AF.Ln)
                        recip = p2sb.tile([1, SQ], F32R, tag="recip")
                        nc.scalar.activation(recip[:], logs[:], AF.Exp,
                                             scale=-1.0)
                        rep_ps = p2rep.tile([P, SQ], F32, tag="repps")
                        dummy_mm(rep_ps)
                        nc.tensor.matmul(rep_ps[:], onesrow_r[:], recip[:],
                                         start=True, stop=True)
                        rep_sb = p2sb.tile([P, SQ], F32, tag="repsb")
                        nc.vector.tensor_copy(rep_sb[:], rep_ps[:])
                        nc.vector.tensor_mul(attnT[:, h, qs:qs + SQ],
                                             at_ps[:], rep_sb[:])

            # ================= phase 3: o_proj partial =====================
            with tc.tile_pool(name="p3sb", bufs=2) as p3sb, \
                 tc.tile_pool(name="p3ps", bufs=4, space="PSUM") as p3ps:
                for ot in range(KO):
                    stage = p3sb.tile([P, S], F32, tag="p3stage")
                    dve_war_touch(stage)
                    for sc in range(NQC):
                        ps = p3ps.tile([P, SQ], F32, tag="p3ps")
                        dummy_mm(ps)
                        for kb in range(NH_LOCAL):
                            nc.tensor.matmul(
                                ps[:], wo_r[:, kb, ot * P:(ot + 1) * P],
                                attnT[:, kb, sc * SQ:(sc + 1) * SQ],
                                start=(kb == 0), stop=(kb == NH_LOCAL - 1))
                        nc.vector.tensor_copy(stage[:, sc * SQ:(sc + 1) * SQ],
                                              ps[:])
                        nc.sync.dma_start(
                            outt.ap()[:, ot, sc * SQ:(sc + 1) * SQ],
                            stage[:, sc * SQ:(sc + 1) * SQ])
    _split_multi_waits(nc)
    return nc


_NC_CACHE = None


def _get_nc():
    global _NC_CACHE
    if _NC_CACHE is None:
        _NC_CACHE = build()
    return _NC_CACHE


def _prep_inputs(hidden_states, w_qkv, w_o):
    """Host-side shard + pre-tile + bf16-cast for the 8 cores."""
    import ml_dtypes
    BF_NP = ml_dtypes.bfloat16
    hidden_states = np.asarray(hidden_states, dtype=np.float32)
    w_qkv = np.asarray(w_qkv, dtype=np.float32)
    w_o = np.asarray(w_o, dtype=np.float32)
    B = hidden_states.shape[0]

    in_maps = []
    xt_by_b = {}
    for b in range(B):
        # xt[p, ko, s] = hidden[b, s, ko*128+p]
        xt = np.ascontiguousarray(
            hidden_states[b].T.reshape(KO, P, S).transpose(1, 0, 2)
        ).astype(BF_NP)
        xt_by_b[b] = xt
    for c in range(8):
        b = c // 4
        hs = [4 * (c % 4) + j for j in range(NH_LOCAL)]
        q_rows = np.concatenate([np.arange(h * P, (h + 1) * P) for h in hs])
        k_rows = q_rows + H
        v_rows = q_rows + 2 * H

        def wtile(rows):
            # [p, ko, o] = w_qkv[rows[o], ko*128+p]
            w = w_qkv[rows, :]                      # [512, 2048]
            return np.ascontiguousarray(
                w.T.reshape(KO, P, len(rows)).transpose(1, 0, 2)
            ).astype(BF_NP)

        # wo[p, kb, o] = w_o[o, cols[kb*128+p]]
        wo_c = np.ascontiguousarray(
            w_o[:, q_rows].T.reshape(NH_LOCAL, P, S).transpose(1, 0, 2)
        ).astype(BF_NP)
        in_maps.append({
            "xt": xt_by_b[b],
            "wq": wtile(q_rows),
            "wk": wtile(k_rows),
            "wv": wtile(v_rows),
            "wo": wo_c,
        })
    return in_maps


def run(hidden_states, w_qkv, w_o, trace=False, trace_cores=None):
    in_maps = _prep_inputs(hidden_states, w_qkv, w_o)
    nc = _get_nc()
    kwargs = {}
    if trace:
        kwargs["trace_cores"] = (trace_cores if trace_cores is not None
                                 else list(range(8)))
    res = run_bass_kernel_spmd(nc, in_maps, core_ids=list(range(8)),
                               trace=trace, **kwargs)
    B, S_, H_ = np.asarray(hidden_states).shape
    out = np.zeros((B, S_, H_), dtype=np.float32)
    for c in range(8):
        b = c // 4
        outt = res.results[c]["outt"]               # [128, 16, 2048]
        outT = outt.transpose(1, 0, 2).reshape(H_, S_)
        out[b] += outT.T
    return out, res


def kernel(hidden_states, w_qkv, w_o):
    out, _ = run(hidden_states, w_qkv, w_o, trace=False)
    return out
